# revision 2
# baseline (speedup 1.0000x reference)
"""
DistanceSampling Trainium2 kernel (8 NeuronCores, SPMD over patch rows).

Computation per 2x2/stride-2 patch of x (1, 256, 512, 512) fp32:
  mean over the 4 patch elements (per channel), d_k = ||x_k - mean + eps||_2
  over channels, k* = argmax_k d_k (first occurrence), out = x_{k*}.
Output: (1, 256, 65536) fp32.

Sharding: core m gets image rows [64m, 64m+64) = 32 patch rows = 8192 patch
locations; fully independent, no collectives. Output chunks concatenated on
the host along L.

Per-core design, processed in 18 units of 1-2 patch rows (fine first/last
units shrink pipeline fill/drain; 2*na image rows, na*256 locations each):
  - channels on SBUF partitions (2 blocks of 128 = cb), locations on the
    free dim; X keeps the natural interleaved layout a*1024+h*512+f*2+s.
  - GpSimd: one pair-add (x_{h,s=0}+x_{h,s=1} for both h) per cb, and the
    h=0 half of D = x - w.  DVE: v = p+r, the h=1 half of D, and the 3
    predicated selects per cb.  ScalarE: w = v/4 - eps (fused scale+bias,
    so D_k = x_k - mean + eps exactly), S = D^2, the select-init copy, the
    PSUM->SBUF distance copy, and both Sign steps of the argmax network.
  - distances: dist_k = sum_c S on PE with one-hot fp32 weight columns
    (8 accumulating matmuls into one [4, L] PSUM tile).  fp32 is required:
    f32r (~13 mantissa bits) flips 46 argmaxes = rel err 3.8e-2 > 2e-2.
  - argmax as matmuls: pairwise differences (A: 4->6 rows, fp32), u =
    Sign(diff) on ScalarE, beats-count with an extra zero column
    (M: 6->5 rows, bf16), m = Sign(beats - 2.5) in {-1,+1} with row 4 =
    Sign(0+1) = 1, masks = SEL5 @ m in {0, 2} broadcast to 128 partitions
    (3 matmuls); exact first-occurrence semantics, ties fall back to x0.
  - selection: o = x0 (ScalarE copy), 3 copy_predicated overwrites (DVE),
    DMA out.  PSUM: dist bufs=3 / diff+beats shared tile bufs=2 / one
    3-bank mask tile (8 banks total).

All distance math is fp32; argmax matches the reference exactly on the
test input (0 flipped locations, rel err 0.0).  Measured 244 us per 8-core
dispatch (v1 baseline 268 us; engine busy ~176 us PE / 175 DVE / 153
GpSimd / 140 Act; DMA ~154 us; overlap ~72%).
"""

import sys

sys.path.insert(0, "/opt/trn_rl_repo")

import numpy as np

import concourse.bacc as bacc
import concourse.bass as bass
import concourse.mybir as mybir
import concourse.tile as tile
from concourse.bass_utils import run_bass_kernel_spmd

f32 = mybir.dt.float32
bf16 = mybir.dt.bfloat16
i32 = mybir.dt.int32
Alu = mybir.AluOpType
Act = mybir.ActivationFunctionType

EPS = 1e-6
C, H, W = 256, 512, 512
NCORES = 8
RPC = H // NCORES  # image rows per core (64)
QPC = RPC // 2  # patch rows per core (32)
QP = QPC // 2  # qpair groups per core (16)
FW = W // 2  # patches per row (256)
LPC = QPC * FW  # locations per core (8192)


def _kernel_body(tc):
    nc = tc.nc
    x = nc.dram_tensor("x", [C, RPC, W], f32, kind="ExternalInput").ap()
    cE = nc.dram_tensor("cE", [128, 16], f32, kind="ExternalInput").ap()
    cA = nc.dram_tensor("cA", [4, 6], f32, kind="ExternalInput").ap()
    cM = nc.dram_tensor("cM", [6, 5], bf16, kind="ExternalInput").ap()
    cNEG = nc.dram_tensor("cNEG", [5, 1], f32, kind="ExternalInput").ap()
    cSEL = nc.dram_tensor("cSEL", [5, 384], bf16, kind="ExternalInput").ap()
    out = nc.dram_tensor("out", [C, LPC], f32, kind="ExternalOutput").ap()

    with (
        tc.tile_pool(name="const", bufs=1) as constp,
        tc.tile_pool(name="xin", bufs=4) as xp,
        tc.tile_pool(name="mid", bufs=3) as mp,
        tc.tile_pool(name="dsq", bufs=2) as dp,
        tc.tile_pool(name="small", bufs=2) as smp,
        tc.tile_pool(name="sel", bufs=3) as sp,
        tc.tile_pool(name="ps_dist", bufs=3, space=bass.MemorySpace.PSUM) as pd,
        tc.tile_pool(name="ps_sm", bufs=2, space=bass.MemorySpace.PSUM) as pb,
        tc.tile_pool(name="ps_mask", bufs=1, space=bass.MemorySpace.PSUM) as pm,
    ):
        E = constp.tile([128, 16], f32)
        nc.sync.dma_start(E[:], cE)
        A = constp.tile([4, 6], f32)
        nc.sync.dma_start(A[:], cA)
        M = constp.tile([6, 5], bf16)
        nc.sync.dma_start(M[:], cM)
        NEG = constp.tile([5, 1], f32)
        nc.sync.dma_start(NEG[:], cNEG)
        SEL = constp.tile([5, 384], bf16)
        nc.sync.dma_start(SEL[:], cSEL)


        def process(u0, na):
            """Handle patch rows [u0, u0+na) (na in {1,2}); 2*na image rows,
            na*256 locations starting at column u0*256 of out."""
            L = na * 256  # locations
            dist_full = pd.tile([4, 512], f32, tag="dist_ps")
            dist_ps = dist_full[:, :L]
            Xs = []
            for cb in range(2):
                X = xp.tile([128, 2048], f32, tag=f"X{cb}")
                nc.sync.dma_start(
                    X[:, : na * 1024],
                    x[cb * 128 : (cb + 1) * 128, 2 * u0 : 2 * (u0 + na), :],
                )
                Xs.append(X)
                # X free layout: a*1024 + h*512 + f*2 + s
                xv = X[:, : na * 1024].rearrange(
                    "p (a h f s) -> p a h f s", a=na, h=2, s=2
                )

                pr_t = mp.tile([128, 1024], f32, tag=f"pr{cb}")
                prv = pr_t[:, : na * 512].rearrange(
                    "p (a h f) -> p a h f", a=na, h=2
                )
                nc.gpsimd.tensor_tensor(
                    prv, xv[:, :, :, :, 0], xv[:, :, :, :, 1], Alu.add
                )
                v_t = mp.tile([128, 512], f32, tag=f"v{cb}")
                vv = v_t[:, :L].rearrange("p (a f) -> p a f", a=na)
                nc.vector.tensor_tensor(vv, prv[:, :, 0, :], prv[:, :, 1, :], Alu.add)
                w_t = mp.tile([128, 512], f32, tag=f"w{cb}")
                nc.scalar.activation(
                    w_t[:, :L], v_t[:, :L], Act.Copy, bias=-EPS, scale=0.25
                )
                wv = w_t[:, :L].rearrange("p (a f) -> p a f", a=na)
                wb = wv.unsqueeze(3).broadcast_to([128, na, 256, 2])  # (a,f,s)

                # D in the same interleaved layout as X (contiguous writes)
                D = dp.tile([128, 2048], f32, tag=f"D{cb}")
                dv = D[:, : na * 1024].rearrange(
                    "p (a h f s) -> p a h f s", a=na, h=2, s=2
                )
                # h=0 rows (k0/k1) on Pool, h=1 rows (k2/k3) on DVE
                nc.gpsimd.tensor_tensor(
                    dv[:, :, 0, :, :], xv[:, :, 0, :, :], wb, Alu.subtract
                )
                nc.vector.tensor_tensor(
                    dv[:, :, 1, :, :], xv[:, :, 1, :, :], wb, Alu.subtract
                )

                S = dp.tile([128, 2048], f32, tag=f"S{cb}")
                nc.scalar.activation(S[:, : na * 1024], D[:, : na * 1024], Act.Square)
                sv = S[:, : na * 1024].rearrange(
                    "p (a h f s) -> p a h f s", a=na, h=2, s=2
                )
                for k in range(4):
                    nc.tensor.matmul(
                        dist_ps,
                        E[:, 4 * k : 4 * k + 4],
                        sv[:, :, k // 2, :, k % 2],
                        start=(cb == 0 and k == 0),
                        stop=(cb == 1 and k == 3),
                    )

            # ------------- stage B: argmax masks -------------------------
            dist4 = smp.tile([4, 512], f32, tag="dist4")
            nc.scalar.copy(dist4[:, :L], dist_ps)
            small_ps = pb.tile([6, 512], f32, tag="small_ps")
            diff_ps = small_ps[:, :L]
            nc.tensor.matmul(diff_ps, A[:], dist4[:, :L])
            u_sb = smp.tile([6, 512], bf16, tag="u_sb")
            nc.scalar.activation(u_sb[:, :L], diff_ps, Act.Sign)
            b_ps = small_ps[0:5, :L]
            nc.tensor.matmul(b_ps, M[:], u_sb[:, :L])
            # rows 0-3: +1 where b==3 (first-occurrence argmax), else -1;
            # row 4: Sign(0 + 1) = 1 (the constant row for the {0,2} shift)
            m_sb = smp.tile([5, 512], bf16, tag="m_sb")
            nc.scalar.activation(m_sb[0:5, :L], b_ps, Act.Sign, bias=NEG[:])
            mask_ps = pm.tile([128, 1536], f32, tag="mask")
            for g in range(3):
                nc.tensor.matmul(
                    mask_ps[:, g * 512 : g * 512 + L],
                    SEL[:, g * 128 : (g + 1) * 128],
                    m_sb[:, :L],
                    skip_group_check=(g > 0),
                )

            # ------------- selection + output ----------------------------
            for cb in range(2):
                xv = Xs[cb][:, : na * 1024].rearrange(
                    "p (a h f s) -> p a h f s", a=na, h=2, s=2
                )
                o_t = sp.tile([128, 512], f32, tag=f"o{cb}")
                ov = o_t[:, :L].rearrange("p (a f) -> p a f", a=na)
                nc.scalar.copy(ov, xv[:, :, 0, :, 0])
                for g, (hk, sk) in enumerate(((0, 1), (1, 0), (1, 1))):
                    mi = (
                        mask_ps[:, g * 512 : g * 512 + L]
                        .bitcast(i32)
                        .rearrange("p (a f) -> p a f", a=na)
                    )
                    nc.vector.copy_predicated(ov, mi, xv[:, :, hk, :, sk])
                nc.sync.dma_start(
                    out[cb * 128 : (cb + 1) * 128, u0 * 256 : (u0 + na) * 256],
                    o_t[:, :L],
                )

        # half-size first/last iterations shrink pipeline fill/drain
        units = [1, 1] + [2] * (QPC // 2 - 2) + [1, 1]
        u0 = 0
        for na in units:
            process(u0, na)
            u0 += na
        assert u0 == QPC


def _const_arrays():
    import ml_dtypes

    A = np.zeros((4, 6), np.float32)
    for j, (ka, kb) in enumerate(((1, 0), (2, 0), (2, 1), (3, 0), (3, 1), (3, 2))):
        A[ka, j] = 1.0
        A[kb, j] = -1.0
    M = np.array(
        [
            [-1, 1, 0, 0, 0],
            [-1, 0, 1, 0, 0],
            [0, -1, 1, 0, 0],
            [-1, 0, 0, 1, 0],
            [0, -1, 0, 1, 0],
            [0, 0, -1, 1, 0],
        ],
        np.float32,
    ).astype(ml_dtypes.bfloat16)
    NEG = np.array([[-2.5]] * 4 + [[1.0]], np.float32)
    SEL = np.zeros((5, 384), np.float32)
    for g, k in enumerate((1, 2, 3)):
        SEL[k, g * 128 : (g + 1) * 128] = 1.0
    SEL[4, :] = 1.0
    SEL = SEL.astype(ml_dtypes.bfloat16)
    Earr = np.zeros((128, 16), np.float32)
    for k in range(4):
        Earr[:, 4 * k + k] = 1.0
    return {"cA": A, "cM": M, "cSEL": SEL, "cE": Earr, "cNEG": NEG}


_compiled_nc = None


def _get_compiled():
    global _compiled_nc
    if _compiled_nc is None:
        nc = bacc.Bacc(
            "TRN2", target_bir_lowering=False, debug=False, num_devices=NCORES
        )
        with tile.TileContext(nc) as tc:
            _kernel_body(tc)
        nc.compile()
        _compiled_nc = nc
    return _compiled_nc


def run_sharded(x_full: np.ndarray, **spmd_kwargs):
    nc = _get_compiled()
    xs = x_full[0]  # (C, H, W)
    consts = _const_arrays()
    in_maps = [
        {"x": np.ascontiguousarray(xs[:, m * RPC : (m + 1) * RPC, :]), **consts}
        for m in range(NCORES)
    ]
    raw = run_bass_kernel_spmd(nc, in_maps, list(range(NCORES)), **spmd_kwargs)
    outs = [raw.results[m]["out"] for m in range(NCORES)]  # (C, LPC) each
    full = np.concatenate(outs, axis=1)[None]  # (1, C, L)
    return full, raw


def kernel(x: np.ndarray) -> np.ndarray:
    x = np.asarray(x, dtype=np.float32)
    assert x.shape == (1, C, H, W), x.shape
    full, _ = run_sharded(x)
    return full
